# revision 1
# baseline (speedup 1.0000x reference)
"""LSTM decoder (teacher_forcing_ratio=0) on 8 TRN2 NeuronCores.

Strategy
--------
Tensor-parallel over the hidden/gate dimension (each core owns a 256-row
hidden slice = 1024 of the 8192 gate rows), with all state kept transposed
(batch on the SBUF free axis).  The autoregressive feedback
``x_{t+1} = h_t @ W_out.T + b_out`` is folded into the recurrence:

    gates_t = h_{t-1} @ (W_hh + W_ih @ W_out).T + (b + W_ih @ b_out)

so the only cross-core dependency per step is an AllGather of the 256-row
h-slices, and the output projection ``out_t = h_t @ W_out.T`` runs off the
critical path, one step behind.  The batch is split into two half-batch
pipelines so each half's gather chain overlaps the other half's matmuls.

The default variant (``pipe5``) computes the output projection as a
per-core partial from the LOCAL bf16 h-slice (2 matmuls per half instead
of 16 redundant gathered-h ones), with the cross-core reduction done on
the host from the 8 per-core outputs; since the gathered state then feeds
only the gate matmul, the exchange runs in fp8e4 (64KB in / 512KB out per
half), with weights x256 and h x16 to stay clear of the Act-cast
flush-to-zero range (rescaled inside the gate activation).  Gate matmuls
run fp8 x fp8 at the same 1 cycle/row as bf16; the gather-in DMA is split
across both HWDGE queues (SP + Act).  Measured 4.6-5.0ms / rel err
9.3e-3 vs the 8.5ms f32r baseline.  Only ``tgt[:, 0]`` is consumed by
the reference, so just that frame is shipped to the device.
"""

import os

import numpy as np

B, T_FULL, D, H = 512, 128, 128, 2048
NCORES = 8
HS = H // NCORES            # hidden rows per core (256)
MT = (4 * HS) // 128        # gate M-tiles per core (8)
KT = H // 128               # K-slots of the hidden dim (16)

_CACHE = {}


def _build_pipe(t_steps):
    """Half-batch pipelined variant: batch is split into two halves with
    independent recurrences; each half's AllGather overlaps the other
    half's gate matmuls on the PE."""
    import concourse.bacc as bacc
    import concourse.mybir as mybir
    from concourse import tile

    f32 = mybir.dt.float32
    f32r = mybir.dt.float32r
    AF = mybir.ActivationFunctionType
    NB = B // 2  # 256 batch columns per half

    nc = bacc.Bacc("TRN2", target_bir_lowering=False, debug=False,
                   num_devices=NCORES)

    w_eff = nc.dram_tensor("w_eff", [128, KT * MT * 128], f32r, kind="ExternalInput")
    w_ih = nc.dram_tensor("w_ih", [128, MT * 128], f32r, kind="ExternalInput")
    w_out = nc.dram_tensor("w_out", [128, KT * D], f32r, kind="ExternalInput")
    x0t = nc.dram_tensor("x0t", [128, B], f32r, kind="ExternalInput")
    b0 = nc.dram_tensor("b0", [128, MT], f32, kind="ExternalInput")
    beff = nc.dram_tensor("beff", [128, MT], f32, kind="ExternalInput")
    bout = nc.dram_tensor("bout", [128, 1], f32, kind="ExternalInput")
    out_d = nc.dram_tensor("out", [t_steps, D, B], f32, kind="ExternalOutput")
    inb = [nc.dram_tensor(f"inb{h}", [2 * 128, NB], f32r) for h in range(2)]
    outb = [nc.dram_tensor(f"outb{h}", [KT * 128, NB], f32r,
                           addr_space="Shared") for h in range(2)]

    rg = [list(range(NCORES))]

    with tile.TileContext(nc) as tc:
        with (
            tc.tile_pool(name="w", bufs=1) as wp,
            tc.tile_pool(name="st", bufs=1) as sp,
            tc.tile_pool(name="ot", bufs=3) as op_,
            tc.tile_pool(name="ps", bufs=6, space="PSUM") as ps,
            tc.tile_pool(name="pso", bufs=2, space="PSUM") as pso,
        ):
            w_eff_sb = wp.tile([128, KT * MT * 128], f32r)
            w_ih_sb = wp.tile([128, MT * 128], f32r)
            w_out_sb = wp.tile([128, KT * D], f32r)
            b0_sb = wp.tile([128, MT], f32)
            beff_sb = wp.tile([128, MT], f32)
            bout_sb = wp.tile([128, 1], f32)
            x0_sb = wp.tile([128, B], f32r)

            hT = sp.tile([128, KT * B], f32r)  # both halves interleaved per k
            myh = [sp.tile([128, 2 * NB], f32r, name=f"myh{h}", tag=f"myh{h}") for h in range(2)]
            cst = [sp.tile([128, 2 * NB], f32, name=f"c{h}", tag=f"c{h}") for h in range(2)]
            sig = [sp.tile([128, MT * NB], f32, name=f"s{h}", tag=f"s{h}") for h in range(2)]
            tnc = [sp.tile([128, 2 * NB], f32, name=f"tn{h}", tag=f"tn{h}") for h in range(2)]
            tmp = [sp.tile([128, 2 * NB], f32, name=f"tm{h}", tag=f"tm{h}") for h in range(2)]

            nc.sync.dma_start(w_eff_sb[:], w_eff[:])
            nc.sync.dma_start(w_ih_sb[:], w_ih[:])
            nc.sync.dma_start(w_out_sb[:], w_out[:])
            nc.sync.dma_start(b0_sb[:], b0[:])
            nc.sync.dma_start(beff_sb[:], beff[:])
            nc.sync.dma_start(bout_sb[:], bout[:])
            nc.sync.dma_start(x0_sb[:], x0t[:])

            def hT_cols(k, h):
                return hT[:, k * B + h * NB:k * B + h * NB + NB]

            def emit_gates(t, h):
                co = h * NB
                for m in range(MT):
                    pt = ps.tile([128, NB], f32, tag="g")
                    if t == 0:
                        nc.tensor.matmul(pt[:], w_ih_sb[:, m * 128:(m + 1) * 128],
                                         x0_sb[:, co:co + NB],
                                         start=True, stop=True)
                    else:
                        for k in range(KT):
                            lhsT = w_eff_sb[:, (k * MT + m) * 128:(k * MT + m + 1) * 128]
                            nc.tensor.matmul(pt[:], lhsT, hT_cols(k, h),
                                             start=(k == 0), stop=(k == KT - 1))
                    func = AF.Tanh if m in (4, 5) else AF.Sigmoid
                    bias = (b0_sb if t == 0 else beff_sb)[:, m:m + 1]
                    nc.scalar.activation(sig[h][:, m * NB:(m + 1) * NB], pt[:],
                                         func, bias=bias)

            def emit_wout(t, h):
                # out_t(half h) = h_t @ W_out.T + b_out; must be emitted while
                # hT still holds h_t for this half (before the next gather).
                po = pso.tile([128, NB], f32, tag="po")
                for k in range(KT):
                    nc.tensor.matmul(po[:], w_out_sb[:, k * D:(k + 1) * D],
                                     hT_cols(k, h),
                                     start=(k == 0), stop=(k == KT - 1))
                ot = op_.tile([128, NB], f32, tag="ot")
                nc.scalar.activation(ot[:], po[:], AF.Identity,
                                     bias=bout_sb[:, 0:1])
                nc.sync.dma_start(out_d[t][:, h * NB:h * NB + NB], ot[:])

            def emit_update_and_gather(t, h):
                for s in range(2):
                    si = sig[h][:, (0 + s) * NB:(1 + s) * NB]
                    sf = sig[h][:, (2 + s) * NB:(3 + s) * NB]
                    tg = sig[h][:, (4 + s) * NB:(5 + s) * NB]
                    so = sig[h][:, (6 + s) * NB:(7 + s) * NB]
                    cs = cst[h][:, s * NB:(s + 1) * NB]
                    if t == 0:
                        nc.vector.tensor_mul(cs, si, tg)
                    else:
                        nc.vector.tensor_mul(cs, sf, cs)
                        nc.vector.tensor_mul(tmp[h][:, s * NB:(s + 1) * NB], si, tg)
                        nc.vector.tensor_add(cs, cs, tmp[h][:, s * NB:(s + 1) * NB])
                    nc.scalar.activation(tnc[h][:, s * NB:(s + 1) * NB], cs, AF.Tanh)
                    nc.vector.tensor_mul(myh[h][:, s * NB:(s + 1) * NB], so,
                                         tnc[h][:, s * NB:(s + 1) * NB])
                nc.sync.dma_start(
                    inb[h].ap().rearrange("(s p) n -> p s n", s=2),
                    myh[h][:].rearrange("p (s n) -> p s n", s=2),
                )
                nc.gpsimd.collective_compute(
                    "AllGather", mybir.AluOpType.bypass, replica_groups=rg,
                    ins=[inb[h].ap().opt()], outs=[outb[h].ap().opt()],
                )
                for k in range(KT):
                    nc.sync.dma_start(
                        hT_cols(k, h),
                        outb[h].ap()[k * 128:(k + 1) * 128, :],
                    )

            for t in range(t_steps):
                emit_gates(t, 0)
                if t > 0:
                    emit_wout(t - 1, 1)   # h_{t-1} half1 still in hT
                emit_update_and_gather(t, 0)
                emit_gates(t, 1)
                emit_wout(t, 0)           # reads h_t half0 (just gathered)
                emit_update_and_gather(t, 1)

            emit_wout(t_steps - 1, 1)

    nc.compile()
    return nc


def _build_pipe2(t_steps, no_ag=False, delay_w=True, split_dma=False):
    """bf16 variant: weights and hidden state in bf16 (validated ~6e-4
    quantization error on host), halving collective + DMA bytes and PE
    stream time vs f32r.  Collective buffers are shaped so every DMA line
    is >=1KB contiguous, and each gather direction is a single coalesced
    DMA.  delay_w: output projection reads one-step-old h via parity
    double-buffered hT so it never waits on the in-flight gather.
    no_ag (diagnostic only): drop the collective instruction to expose the
    pure compute/DMA pipeline floor."""
    import concourse.bacc as bacc
    import concourse.mybir as mybir
    from concourse import tile

    f32 = mybir.dt.float32
    f32r = mybir.dt.float32r
    bf16 = mybir.dt.bfloat16
    AF = mybir.ActivationFunctionType
    NB = B // 2  # 256 batch columns per half

    nc = bacc.Bacc("TRN2", target_bir_lowering=False, debug=False,
                   num_devices=NCORES)

    w_eff = nc.dram_tensor("w_eff", [128, KT * MT * 128], bf16, kind="ExternalInput")
    w_ih = nc.dram_tensor("w_ih", [128, MT * 128], bf16, kind="ExternalInput")
    w_out = nc.dram_tensor("w_out", [128, KT * D], bf16, kind="ExternalInput")
    x0t = nc.dram_tensor("x0t", [128, B], bf16, kind="ExternalInput")
    b0 = nc.dram_tensor("b0", [128, MT], f32, kind="ExternalInput")
    beff = nc.dram_tensor("beff", [128, MT], f32, kind="ExternalInput")
    bout = nc.dram_tensor("bout", [128, 1], f32, kind="ExternalInput")
    out_d = nc.dram_tensor("out", [t_steps, D, B], f32, kind="ExternalOutput")
    # inb[h]: this core's h-slice, [128 partitions, 2*NB] (rows r / r+128 of
    # the 256-row slice side by side).  AllGather concatenates over cores on
    # dim0, so outb[c*128:(c+1)*128, :] holds core c's slice and maps to
    # k-tiles (2c, 2c+1) -- the same (k p) convention the weights use.
    inb = [nc.dram_tensor(f"inb{h}", [128, 2 * NB], bf16) for h in range(2)]
    outb = [nc.dram_tensor(f"outb{h}", [NCORES * 128, 2 * NB], bf16,
                           addr_space="Shared") for h in range(2)]

    rg = [list(range(NCORES))]
    M_ORDER = (0, 1, 4, 5, 2, 3, 6, 7)  # i, g first; f next; o last

    with tile.TileContext(nc) as tc:
        with (
            tc.tile_pool(name="w", bufs=1) as wp,
            tc.tile_pool(name="st", bufs=1) as sp,
            tc.tile_pool(name="ot", bufs=3) as op_,
            tc.tile_pool(name="ps", bufs=6, space="PSUM") as ps,
            tc.tile_pool(name="pso", bufs=2, space="PSUM") as pso,
        ):
            w_eff_sb = wp.tile([128, KT * MT * 128], bf16)
            w_ih_sb = wp.tile([128, MT * 128], bf16)
            w_out_sb = wp.tile([128, KT * D], bf16)
            b0_sb = wp.tile([128, MT], f32)
            beff_sb = wp.tile([128, MT], f32)
            bout_sb = wp.tile([128, 1], f32)
            x0_sb = wp.tile([128, B], bf16)

            hT = [[sp.tile([128, KT * NB], bf16, name=f"hT{h}_{p}",
                            tag=f"hT{h}_{p}") for p in range(2 if delay_w else 1)]
                  for h in range(2)]
            myh = [sp.tile([128, 2 * NB], bf16, name=f"myh{h}", tag=f"myh{h}")
                   for h in range(2)]
            cst = [sp.tile([128, 2 * NB], f32, name=f"c{h}", tag=f"c{h}")
                   for h in range(2)]
            sig = [sp.tile([128, MT * NB], f32, name=f"s{h}", tag=f"s{h}")
                   for h in range(2)]
            tnc = [sp.tile([128, 2 * NB], f32, name=f"tn{h}", tag=f"tn{h}")
                   for h in range(2)]
            tmp = [sp.tile([128, 2 * NB], f32, name=f"tm{h}", tag=f"tm{h}")
                   for h in range(2)]

            nc.sync.dma_start(w_eff_sb[:], w_eff[:])
            nc.sync.dma_start(w_ih_sb[:], w_ih[:])
            nc.sync.dma_start(w_out_sb[:], w_out[:])
            nc.sync.dma_start(b0_sb[:], b0[:])
            nc.sync.dma_start(beff_sb[:], beff[:])
            nc.sync.dma_start(bout_sb[:], bout[:])
            nc.sync.dma_start(x0_sb[:], x0t[:])

            def hTb(t, h):
                return hT[h][t % 2 if delay_w else 0]

            def emit_gates(t, h):
                co = h * NB
                src_hT = hTb(t - 1, h)
                for m in M_ORDER:
                    pt = ps.tile([128, NB], f32, tag="g")
                    if t == 0:
                        nc.tensor.matmul(pt[:], w_ih_sb[:, m * 128:(m + 1) * 128],
                                         x0_sb[:, co:co + NB],
                                         start=True, stop=True)
                    else:
                        for k in range(KT):
                            lhsT = w_eff_sb[:, (k * MT + m) * 128:(k * MT + m + 1) * 128]
                            nc.tensor.matmul(pt[:], lhsT,
                                             src_hT[:, k * NB:(k + 1) * NB],
                                             start=(k == 0), stop=(k == KT - 1))
                    func = AF.Tanh if m in (4, 5) else AF.Sigmoid
                    bias = (b0_sb if t == 0 else beff_sb)[:, m:m + 1]
                    nc.scalar.activation(sig[h][:, m * NB:(m + 1) * NB], pt[:],
                                         func, bias=bias)

            def emit_wout(t, h):
                # out_t(half h) = h_t @ W_out.T + b_out, from gathered hT.
                src_hT = hTb(t, h)
                po = pso.tile([128, NB], f32, tag="po")
                for k in range(KT):
                    nc.tensor.matmul(po[:], w_out_sb[:, k * D:(k + 1) * D],
                                     src_hT[:, k * NB:(k + 1) * NB],
                                     start=(k == 0), stop=(k == KT - 1))
                ot = op_.tile([128, NB], f32, tag="ot")
                nc.scalar.activation(ot[:], po[:], AF.Identity,
                                     bias=bout_sb[:, 0:1])
                if split_dma:
                    nc.scalar.dma_start(out_d[t][:, h * NB:h * NB + NB], ot[:])
                else:
                    nc.sync.dma_start(out_d[t][:, h * NB:h * NB + NB], ot[:])

            def emit_update_and_gather(t, h):
                W = 2 * NB
                si = sig[h][:, 0:W]
                sf = sig[h][:, 2 * NB:2 * NB + W]
                tg = sig[h][:, 4 * NB:4 * NB + W]
                so = sig[h][:, 6 * NB:6 * NB + W]
                cs = cst[h][:, 0:W]
                if t == 0:
                    nc.vector.tensor_mul(cs, si, tg)
                else:
                    nc.vector.tensor_mul(tmp[h][:, 0:W], si, tg)
                    nc.vector.tensor_mul(cs, sf, cs)
                    nc.vector.tensor_add(cs, cs, tmp[h][:, 0:W])
                nc.scalar.activation(tnc[h][:, 0:W], cs, AF.Tanh)
                nc.vector.tensor_mul(myh[h][:, 0:W], so, tnc[h][:, 0:W])
                nc.sync.dma_start(inb[h].ap(), myh[h][:])
                if not no_ag:
                    nc.gpsimd.collective_compute(
                        "AllGather", mybir.AluOpType.bypass, replica_groups=rg,
                        ins=[inb[h].ap().opt()], outs=[outb[h].ap().opt()],
                    )
                if split_dma:
                    # halve the serial gather-in time: one half per HWDGE queue
                    NH = NCORES // 2
                    hc = NH * 2 * NB
                    dst = hTb(t, h)
                    nc.sync.dma_start(
                        dst[:, 0:hc].rearrange("p (c m) -> p c m", c=NH),
                        outb[h].ap()[0:NH * 128, :].rearrange(
                            "(c p) m -> p c m", c=NH),
                    )
                    nc.scalar.dma_start(
                        dst[:, hc:2 * hc].rearrange("p (c m) -> p c m", c=NH),
                        outb[h].ap()[NH * 128:NCORES * 128, :].rearrange(
                            "(c p) m -> p c m", c=NH),
                    )
                else:
                    nc.sync.dma_start(
                        hTb(t, h)[:].rearrange("p (c m) -> p c m", c=NCORES),
                        outb[h].ap().rearrange("(c p) m -> p c m", c=NCORES),
                    )

            for t in range(t_steps):
                emit_gates(t, 0)
                if t > 0:
                    emit_wout(t - 1, 0 if delay_w else 1)
                emit_update_and_gather(t, 0)
                emit_gates(t, 1)
                if delay_w:
                    if t > 0:
                        emit_wout(t - 1, 1)
                else:
                    emit_wout(t, 0)
                emit_update_and_gather(t, 1)

            emit_wout(t_steps - 1, 0 if delay_w else 1)
            if delay_w:
                emit_wout(t_steps - 1, 1)

    nc.compile()
    return nc


def _build_pipe4(t_steps):
    """pipe2s + local output projection: each core computes only the partial
    out contribution of its own 256-row h-slice (2 matmuls per half instead
    of 16 redundant ones); the host sums the 8 per-core partials.  The
    projection reads local myh, so the gathered hT needs no parity
    double-buffering and the gather is consumed by the gates only."""
    import concourse.bacc as bacc
    import concourse.mybir as mybir
    from concourse import tile

    f32 = mybir.dt.float32
    bf16 = mybir.dt.bfloat16
    AF = mybir.ActivationFunctionType
    NB = B // 2  # 256 batch columns per half

    nc = bacc.Bacc("TRN2", target_bir_lowering=False, debug=False,
                   num_devices=NCORES)

    w_eff = nc.dram_tensor("w_eff", [128, KT * MT * 128], bf16, kind="ExternalInput")
    w_ih = nc.dram_tensor("w_ih", [128, MT * 128], bf16, kind="ExternalInput")
    w_own = nc.dram_tensor("w_own", [128, 2 * D], bf16, kind="ExternalInput")
    x0t = nc.dram_tensor("x0t", [128, B], bf16, kind="ExternalInput")
    b0 = nc.dram_tensor("b0", [128, MT], f32, kind="ExternalInput")
    beff = nc.dram_tensor("beff", [128, MT], f32, kind="ExternalInput")
    out_d = nc.dram_tensor("out", [t_steps, D, B], f32, kind="ExternalOutput")
    inb = [nc.dram_tensor(f"inb{h}", [128, 2 * NB], bf16) for h in range(2)]
    outb = [nc.dram_tensor(f"outb{h}", [NCORES * 128, 2 * NB], bf16,
                           addr_space="Shared") for h in range(2)]

    rg = [list(range(NCORES))]
    M_ORDER = (0, 1, 4, 5, 2, 3, 6, 7)  # i, g first; f next; o last

    with tile.TileContext(nc) as tc:
        with (
            tc.tile_pool(name="w", bufs=1) as wp,
            tc.tile_pool(name="st", bufs=1) as sp,
            tc.tile_pool(name="ot", bufs=3) as op_,
            tc.tile_pool(name="ps", bufs=6, space="PSUM") as ps,
            tc.tile_pool(name="pso", bufs=2, space="PSUM") as pso,
        ):
            w_eff_sb = wp.tile([128, KT * MT * 128], bf16)
            w_ih_sb = wp.tile([128, MT * 128], bf16)
            w_own_sb = wp.tile([128, 2 * D], bf16)
            b0_sb = wp.tile([128, MT], f32)
            beff_sb = wp.tile([128, MT], f32)
            x0_sb = wp.tile([128, B], bf16)

            hT = [sp.tile([128, KT * NB], bf16, name=f"hT{h}", tag=f"hT{h}")
                  for h in range(2)]
            myh = [sp.tile([128, 2 * NB], bf16, name=f"myh{h}", tag=f"myh{h}")
                   for h in range(2)]
            cst = [sp.tile([128, 2 * NB], f32, name=f"c{h}", tag=f"c{h}")
                   for h in range(2)]
            sig = [sp.tile([128, MT * NB], f32, name=f"s{h}", tag=f"s{h}")
                   for h in range(2)]
            tnc = [sp.tile([128, 2 * NB], f32, name=f"tn{h}", tag=f"tn{h}")
                   for h in range(2)]
            tmp = [sp.tile([128, 2 * NB], f32, name=f"tm{h}", tag=f"tm{h}")
                   for h in range(2)]

            nc.sync.dma_start(w_eff_sb[:], w_eff[:])
            nc.sync.dma_start(w_ih_sb[:], w_ih[:])
            nc.sync.dma_start(w_own_sb[:], w_own[:])
            nc.sync.dma_start(b0_sb[:], b0[:])
            nc.sync.dma_start(beff_sb[:], beff[:])
            nc.sync.dma_start(x0_sb[:], x0t[:])

            def emit_gates(t, h):
                co = h * NB
                for m in M_ORDER:
                    pt = ps.tile([128, NB], f32, tag="g")
                    if t == 0:
                        nc.tensor.matmul(pt[:], w_ih_sb[:, m * 128:(m + 1) * 128],
                                         x0_sb[:, co:co + NB],
                                         start=True, stop=True)
                    else:
                        for k in range(KT):
                            lhsT = w_eff_sb[:, (k * MT + m) * 128:(k * MT + m + 1) * 128]
                            nc.tensor.matmul(pt[:], lhsT,
                                             hT[h][:, k * NB:(k + 1) * NB],
                                             start=(k == 0), stop=(k == KT - 1))
                    func = AF.Tanh if m in (4, 5) else AF.Sigmoid
                    bias = (b0_sb if t == 0 else beff_sb)[:, m:m + 1]
                    nc.scalar.activation(sig[h][:, m * NB:(m + 1) * NB], pt[:],
                                         func, bias=bias)

            def emit_wout(t, h):
                # partial out: W_out[:, own slice] @ myh (local, bf16); the
                # host sums partials across cores and adds b_out.
                po = pso.tile([128, NB], f32, tag="po")
                for s in range(2):
                    nc.tensor.matmul(po[:], w_own_sb[:, s * D:(s + 1) * D],
                                     myh[h][:, s * NB:(s + 1) * NB],
                                     start=(s == 0), stop=(s == 1))
                ot = op_.tile([128, NB], f32, tag="ot")
                nc.scalar.activation(ot[:], po[:], AF.Identity)
                nc.scalar.dma_start(out_d[t][:, h * NB:h * NB + NB], ot[:])

            def emit_update_and_gather(t, h):
                W = 2 * NB
                si = sig[h][:, 0:W]
                sf = sig[h][:, 2 * NB:2 * NB + W]
                tg = sig[h][:, 4 * NB:4 * NB + W]
                so = sig[h][:, 6 * NB:6 * NB + W]
                cs = cst[h][:, 0:W]
                if t == 0:
                    nc.vector.tensor_mul(cs, si, tg)
                else:
                    nc.vector.tensor_mul(tmp[h][:, 0:W], si, tg)
                    nc.vector.tensor_mul(cs, sf, cs)
                    nc.vector.tensor_add(cs, cs, tmp[h][:, 0:W])
                nc.scalar.activation(tnc[h][:, 0:W], cs, AF.Tanh)
                nc.vector.tensor_mul(myh[h][:, 0:W], so, tnc[h][:, 0:W])
                nc.sync.dma_start(inb[h].ap(), myh[h][:])
                nc.gpsimd.collective_compute(
                    "AllGather", mybir.AluOpType.bypass, replica_groups=rg,
                    ins=[inb[h].ap().opt()], outs=[outb[h].ap().opt()],
                )
                NH = NCORES // 2
                hc = NH * 2 * NB
                nc.sync.dma_start(
                    hT[h][:, 0:hc].rearrange("p (c m) -> p c m", c=NH),
                    outb[h].ap()[0:NH * 128, :].rearrange(
                        "(c p) m -> p c m", c=NH),
                )
                nc.scalar.dma_start(
                    hT[h][:, hc:2 * hc].rearrange("p (c m) -> p c m", c=NH),
                    outb[h].ap()[NH * 128:NCORES * 128, :].rearrange(
                        "(c p) m -> p c m", c=NH),
                )

            for t in range(t_steps):
                emit_gates(t, 0)
                emit_update_and_gather(t, 0)
                emit_wout(t, 0)
                emit_gates(t, 1)
                emit_update_and_gather(t, 1)
                emit_wout(t, 1)

    nc.compile()
    return nc


def _build_pipe5(t_steps, dve_cast=False):
    """pipe4 with an fp8-only exchange: the gathered h feeds only the gate
    matmul (the output projection reads local bf16 h), and the recurrence
    tolerates fp8 h (host sim 9.3e-3 vs the 2e-2 gate with x16 h / x256
    weight scaling keeping values out of the Act-cast flush-to-zero range).
    Payload per half drops to 64KB in / 512KB out, shrinking every hop of
    the per-step gather chain; gate matmuls run fp8 x fp8 at the same
    1 cycle/row as bf16."""
    import concourse.bacc as bacc
    import concourse.mybir as mybir
    from concourse import tile

    f32 = mybir.dt.float32
    bf16 = mybir.dt.bfloat16
    fp8 = mybir.dt.float8e4
    AF = mybir.ActivationFunctionType
    NB = B // 2  # 256 batch columns per half
    WS = 256.0   # fp8 weight scale
    HS = 16.0    # fp8 hidden-state scale

    nc = bacc.Bacc("TRN2", target_bir_lowering=False, debug=False,
                   num_devices=NCORES)

    w_eff8 = nc.dram_tensor("w_eff8", [128, KT * MT * 128], fp8, kind="ExternalInput")
    w_ih = nc.dram_tensor("w_ih", [128, MT * 128], bf16, kind="ExternalInput")
    w_own = nc.dram_tensor("w_own", [128, 2 * D], bf16, kind="ExternalInput")
    x0t = nc.dram_tensor("x0t", [128, B], bf16, kind="ExternalInput")
    b0 = nc.dram_tensor("b0", [128, MT], f32, kind="ExternalInput")
    beff = nc.dram_tensor("beff", [128, MT], f32, kind="ExternalInput")
    out_d = nc.dram_tensor("out", [t_steps, D, B], f32, kind="ExternalOutput")
    inb = [nc.dram_tensor(f"inb{h}", [128, 2 * NB], fp8) for h in range(2)]
    outb = [nc.dram_tensor(f"outb{h}", [NCORES * 128, 2 * NB], fp8,
                           addr_space="Shared") for h in range(2)]

    rg = [list(range(NCORES))]
    M_ORDER = (0, 1, 4, 5, 2, 3, 6, 7)  # i, g first; f next; o last

    with tile.TileContext(nc) as tc:
        with (
            tc.tile_pool(name="w", bufs=1) as wp,
            tc.tile_pool(name="st", bufs=1) as sp,
            tc.tile_pool(name="ot", bufs=3) as op_,
            tc.tile_pool(name="ps", bufs=6, space="PSUM") as ps,
            tc.tile_pool(name="pso", bufs=2, space="PSUM") as pso,
        ):
            w_eff_sb = wp.tile([128, KT * MT * 128], fp8)
            w_ih_sb = wp.tile([128, MT * 128], bf16)
            w_own_sb = wp.tile([128, 2 * D], bf16)
            b0_sb = wp.tile([128, MT], f32)
            beff_sb = wp.tile([128, MT], f32)
            x0_sb = wp.tile([128, B], bf16)

            hT8 = [sp.tile([128, KT * NB], fp8, name=f"h8{h}", tag=f"h8{h}")
                   for h in range(2)]
            myh = [sp.tile([128, 2 * NB], bf16, name=f"myh{h}", tag=f"myh{h}")
                   for h in range(2)]
            myh8 = [sp.tile([128, 2 * NB], fp8, name=f"m8{h}", tag=f"m8{h}")
                    for h in range(2)]
            cst = [sp.tile([128, 2 * NB], f32, name=f"c{h}", tag=f"c{h}")
                   for h in range(2)]
            sig = [sp.tile([128, MT * NB], f32, name=f"s{h}", tag=f"s{h}")
                   for h in range(2)]
            tnc = [sp.tile([128, 2 * NB], f32, name=f"tn{h}", tag=f"tn{h}")
                   for h in range(2)]
            tmp = [sp.tile([128, 2 * NB], f32, name=f"tm{h}", tag=f"tm{h}")
                   for h in range(2)]

            nc.sync.dma_start(w_eff_sb[:], w_eff8[:])
            nc.sync.dma_start(w_ih_sb[:], w_ih[:])
            nc.sync.dma_start(w_own_sb[:], w_own[:])
            nc.sync.dma_start(b0_sb[:], b0[:])
            nc.sync.dma_start(beff_sb[:], beff[:])
            nc.sync.dma_start(x0_sb[:], x0t[:])

            def emit_gates(t, h):
                co = h * NB
                for m in M_ORDER:
                    pt = ps.tile([128, NB], f32, tag="g")
                    if t == 0:
                        nc.tensor.matmul(pt[:], w_ih_sb[:, m * 128:(m + 1) * 128],
                                         x0_sb[:, co:co + NB],
                                         start=True, stop=True)
                    else:
                        for k in range(KT):
                            lhsT = w_eff_sb[:, (k * MT + m) * 128:(k * MT + m + 1) * 128]
                            nc.tensor.matmul(pt[:], lhsT,
                                             hT8[h][:, k * NB:(k + 1) * NB],
                                             start=(k == 0), stop=(k == KT - 1))
                    func = AF.Tanh if m in (4, 5) else AF.Sigmoid
                    bias = (b0_sb if t == 0 else beff_sb)[:, m:m + 1]
                    scale = 1.0 if t == 0 else 1.0 / (WS * HS)
                    nc.scalar.activation(sig[h][:, m * NB:(m + 1) * NB], pt[:],
                                         func, bias=bias, scale=scale)

            def emit_wout(t, h):
                # partial out from the LOCAL bf16 h-slice; host sums cores.
                po = pso.tile([128, NB], f32, tag="po")
                for s in range(2):
                    nc.tensor.matmul(po[:], w_own_sb[:, s * D:(s + 1) * D],
                                     myh[h][:, s * NB:(s + 1) * NB],
                                     start=(s == 0), stop=(s == 1))
                ot = op_.tile([128, NB], f32, tag="ot")
                nc.scalar.activation(ot[:], po[:], AF.Identity)
                nc.scalar.dma_start(out_d[t][:, h * NB:h * NB + NB], ot[:])

            def emit_update_and_gather(t, h):
                W = 2 * NB
                si = sig[h][:, 0:W]
                sf = sig[h][:, 2 * NB:2 * NB + W]
                tg = sig[h][:, 4 * NB:4 * NB + W]
                so = sig[h][:, 6 * NB:6 * NB + W]
                cs = cst[h][:, 0:W]
                if t == 0:
                    nc.vector.tensor_mul(cs, si, tg)
                else:
                    nc.vector.tensor_mul(tmp[h][:, 0:W], si, tg)
                    nc.vector.tensor_mul(cs, sf, cs)
                    nc.vector.tensor_add(cs, cs, tmp[h][:, 0:W])
                nc.scalar.activation(tnc[h][:, 0:W], cs, AF.Tanh)
                nc.vector.tensor_mul(myh[h][:, 0:W], so, tnc[h][:, 0:W])
                if dve_cast:
                    # fused (so*16)*tanh(c) straight to fp8 on the DVE --
                    # keeps the Act queue out of the gather chain
                    nc.vector.scalar_tensor_tensor(
                        myh8[h][:], so, HS, tnc[h][:, 0:W],
                        mybir.AluOpType.mult, mybir.AluOpType.mult)
                else:
                    nc.scalar.activation(myh8[h][:], myh[h][:], AF.Copy,
                                         scale=HS)
                nc.sync.dma_start(inb[h].ap(), myh8[h][:])
                nc.gpsimd.collective_compute(
                    "AllGather", mybir.AluOpType.bypass, replica_groups=rg,
                    ins=[inb[h].ap().opt()], outs=[outb[h].ap().opt()],
                )
                NH = NCORES // 2
                hc = NH * 2 * NB
                nc.sync.dma_start(
                    hT8[h][:, 0:hc].rearrange("p (c m) -> p c m", c=NH),
                    outb[h].ap()[0:NH * 128, :].rearrange(
                        "(c p) m -> p c m", c=NH),
                )
                nc.scalar.dma_start(
                    hT8[h][:, hc:2 * hc].rearrange("p (c m) -> p c m", c=NH),
                    outb[h].ap()[NH * 128:NCORES * 128, :].rearrange(
                        "(c p) m -> p c m", c=NH),
                )

            for t in range(t_steps):
                emit_gates(t, 0)
                emit_update_and_gather(t, 0)
                emit_wout(t, 0)
                emit_gates(t, 1)
                emit_update_and_gather(t, 1)
                emit_wout(t, 1)

    nc.compile()
    return nc


def _build_pipe3(t_steps):
    """fp8 DoubleRow gates on top of pipe2: the recurrence matmul runs with
    fp8e4 weights (x64 scale) and fp8e4 hidden state in DoubleRow perf mode
    (2 K-rows per PE pass), halving PE time.  The output projection keeps
    reading bf16 h (fp8 h there fails tolerance), so the collective carries
    a packed payload: bf16 h (1KB/part) + fp8 h (0.5KB/part) produced by an
    Act-engine copy, gathered in one AllGather.  Host-simulated worst-case
    rel err ~1e-2 vs the 2e-2 gate."""
    import concourse.bacc as bacc
    import concourse.mybir as mybir
    from concourse import tile

    f32 = mybir.dt.float32
    bf16 = mybir.dt.bfloat16
    fp8 = mybir.dt.float8e4
    AF = mybir.ActivationFunctionType
    DR = mybir.MatmulPerfMode.DoubleRow
    NB = B // 2          # 256 batch columns per half
    JP = KT // 2         # 8 k-pairs (K=256 per DoubleRow matmul)
    WS = 256.0           # fp8 weight scale
    HS = 16.0            # fp8 hidden-state scale (lifts h out of the
                         # flush-to-zero subnormal range of the Act cast)
    PACK = 2 * NB + NB   # 768 bf16 cols: 512 bf16 h + 512 fp8 h bytes

    nc = bacc.Bacc("TRN2", target_bir_lowering=False, debug=False,
                   num_devices=NCORES)

    w_eff8 = nc.dram_tensor("w_eff8", [128, JP * MT * 256], fp8, kind="ExternalInput")
    w_ih = nc.dram_tensor("w_ih", [128, MT * 128], bf16, kind="ExternalInput")
    w_out = nc.dram_tensor("w_out", [128, KT * D], bf16, kind="ExternalInput")
    x0t = nc.dram_tensor("x0t", [128, B], bf16, kind="ExternalInput")
    b0 = nc.dram_tensor("b0", [128, MT], f32, kind="ExternalInput")
    beff = nc.dram_tensor("beff", [128, MT], f32, kind="ExternalInput")
    bout = nc.dram_tensor("bout", [128, 1], f32, kind="ExternalInput")
    out_d = nc.dram_tensor("out", [t_steps, D, B], f32, kind="ExternalOutput")
    inb = [nc.dram_tensor(f"inb{h}", [128, PACK], bf16) for h in range(2)]
    outb = [nc.dram_tensor(f"outb{h}", [NCORES * 128, PACK], bf16,
                           addr_space="Shared") for h in range(2)]

    rg = [list(range(NCORES))]
    M_ORDER = (0, 1, 4, 5, 2, 3, 6, 7)  # i, g first; f next; o last

    with tile.TileContext(nc) as tc:
        with (
            tc.tile_pool(name="w", bufs=1) as wp,
            tc.tile_pool(name="st", bufs=1) as sp,
            tc.tile_pool(name="ot", bufs=3) as op_,
            tc.tile_pool(name="ps", bufs=6, space="PSUM") as ps,
            tc.tile_pool(name="pso", bufs=2, space="PSUM") as pso,
        ):
            w_eff_sb = wp.tile([128, JP * MT * 256], fp8)
            w_ih_sb = wp.tile([128, MT * 128], bf16)
            w_out_sb = wp.tile([128, KT * D], bf16)
            b0_sb = wp.tile([128, MT], f32)
            beff_sb = wp.tile([128, MT], f32)
            bout_sb = wp.tile([128, 1], f32)
            x0_sb = wp.tile([128, B], bf16)

            hT = [[sp.tile([128, KT * NB], bf16, name=f"hT{h}_{p}",
                           tag=f"hT{h}_{p}") for p in range(2)]
                  for h in range(2)]
            hT8 = [[sp.tile([128, KT * NB], fp8, name=f"h8{h}_{p}",
                            tag=f"h8{h}_{p}") for p in range(2)]
                   for h in range(2)]
            myh = [sp.tile([128, PACK], bf16, name=f"myh{h}", tag=f"myh{h}")
                   for h in range(2)]
            cst = [sp.tile([128, 2 * NB], f32, name=f"c{h}", tag=f"c{h}")
                   for h in range(2)]
            sig = [sp.tile([128, MT * NB], f32, name=f"s{h}", tag=f"s{h}")
                   for h in range(2)]
            tnc = [sp.tile([128, 2 * NB], f32, name=f"tn{h}", tag=f"tn{h}")
                   for h in range(2)]
            tmp = [sp.tile([128, 2 * NB], f32, name=f"tm{h}", tag=f"tm{h}")
                   for h in range(2)]

            nc.sync.dma_start(w_eff_sb[:], w_eff8[:])
            nc.sync.dma_start(w_ih_sb[:], w_ih[:])
            nc.sync.dma_start(w_out_sb[:], w_out[:])
            nc.sync.dma_start(b0_sb[:], b0[:])
            nc.sync.dma_start(beff_sb[:], beff[:])
            nc.sync.dma_start(bout_sb[:], bout[:])
            nc.sync.dma_start(x0_sb[:], x0t[:])

            def emit_gates(t, h):
                co = h * NB
                src8 = hT8[h][(t - 1) % 2]
                for m in M_ORDER:
                    pt = ps.tile([128, NB], f32, tag="g")
                    if t == 0:
                        nc.tensor.matmul(pt[:], w_ih_sb[:, m * 128:(m + 1) * 128],
                                         x0_sb[:, co:co + NB],
                                         start=True, stop=True)
                    else:
                        for j in range(JP):
                            lhsT = w_eff_sb[:, (j * MT + m) * 256:
                                            (j * MT + m + 1) * 256].rearrange(
                                "p (two mc) -> p two mc", two=2)
                            rhs = src8[:, 2 * j * NB:(2 * j + 2) * NB].rearrange(
                                "p (two n) -> p two n", two=2)
                            nc.tensor.matmul(pt[:], lhsT, rhs,
                                             start=(j == 0), stop=(j == JP - 1),
                                             perf_mode=DR)
                    func = AF.Tanh if m in (4, 5) else AF.Sigmoid
                    bias = (b0_sb if t == 0 else beff_sb)[:, m:m + 1]
                    scale = 1.0 if t == 0 else 1.0 / (WS * HS)
                    nc.scalar.activation(sig[h][:, m * NB:(m + 1) * NB], pt[:],
                                         func, bias=bias, scale=scale)

            def emit_wout(t, h):
                # out_t = h_t @ W_out.T + b_out from the bf16 gathered h --
                # fp8 h here would blow the error budget.
                src_hT = hT[h][t % 2]
                po = pso.tile([128, NB], f32, tag="po")
                for k in range(KT):
                    nc.tensor.matmul(po[:], w_out_sb[:, k * D:(k + 1) * D],
                                     src_hT[:, k * NB:(k + 1) * NB],
                                     start=(k == 0), stop=(k == KT - 1))
                ot = op_.tile([128, NB], f32, tag="ot")
                nc.scalar.activation(ot[:], po[:], AF.Identity,
                                     bias=bout_sb[:, 0:1])
                nc.sync.dma_start(out_d[t][:, h * NB:h * NB + NB], ot[:])

            def emit_update_and_gather(t, h):
                W = 2 * NB
                si = sig[h][:, 0:W]
                sf = sig[h][:, 2 * NB:2 * NB + W]
                tg = sig[h][:, 4 * NB:4 * NB + W]
                so = sig[h][:, 6 * NB:6 * NB + W]
                cs = cst[h][:, 0:W]
                if t == 0:
                    nc.vector.tensor_mul(cs, si, tg)
                else:
                    nc.vector.tensor_mul(tmp[h][:, 0:W], si, tg)
                    nc.vector.tensor_mul(cs, sf, cs)
                    nc.vector.tensor_add(cs, cs, tmp[h][:, 0:W])
                nc.scalar.activation(tnc[h][:, 0:W], cs, AF.Tanh)
                nc.vector.tensor_mul(myh[h][:, 0:W], so, tnc[h][:, 0:W])
                # fp8 copy packed into the same payload (Act engine cast)
                nc.scalar.activation(
                    myh[h][:, W:W + NB].bitcast(fp8), myh[h][:, 0:W],
                    AF.Copy, scale=HS)
                nc.sync.dma_start(inb[h].ap(), myh[h][:])
                nc.gpsimd.collective_compute(
                    "AllGather", mybir.AluOpType.bypass, replica_groups=rg,
                    ins=[inb[h].ap().opt()], outs=[outb[h].ap().opt()],
                )
                par = t % 2
                nc.sync.dma_start(
                    hT[h][par][:].rearrange("p (c m) -> p c m", c=NCORES),
                    outb[h].ap()[:, 0:W].rearrange("(c p) m -> p c m", c=NCORES),
                )
                nc.sync.dma_start(
                    hT8[h][par][:].rearrange("p (c m) -> p c m", c=NCORES),
                    outb[h].ap()[:, W:W + NB].bitcast(fp8).rearrange(
                        "(c p) m -> p c m", c=NCORES),
                )

            for t in range(t_steps):
                emit_gates(t, 0)
                if t > 0:
                    emit_wout(t - 1, 0)
                emit_update_and_gather(t, 0)
                emit_gates(t, 1)
                if t > 0:
                    emit_wout(t - 1, 1)
                emit_update_and_gather(t, 1)

            emit_wout(t_steps - 1, 0)
            emit_wout(t_steps - 1, 1)

    nc.compile()
    return nc


def _build(t_steps):
    import concourse.bacc as bacc
    import concourse.mybir as mybir
    from concourse import tile

    f32 = mybir.dt.float32
    f32r = mybir.dt.float32r
    AF = mybir.ActivationFunctionType

    nc = bacc.Bacc("TRN2", target_bir_lowering=False, debug=False,
                   num_devices=NCORES)

    w_eff = nc.dram_tensor("w_eff", [128, KT * MT * 128], f32r, kind="ExternalInput")
    w_ih = nc.dram_tensor("w_ih", [128, MT * 128], f32r, kind="ExternalInput")
    w_out = nc.dram_tensor("w_out", [128, KT * D], f32r, kind="ExternalInput")
    x0t = nc.dram_tensor("x0t", [128, B], f32r, kind="ExternalInput")
    b0 = nc.dram_tensor("b0", [128, MT], f32, kind="ExternalInput")
    beff = nc.dram_tensor("beff", [128, MT], f32, kind="ExternalInput")
    bout = nc.dram_tensor("bout", [128, 1], f32, kind="ExternalInput")
    out_d = nc.dram_tensor("out", [t_steps, D, B], f32, kind="ExternalOutput")
    inb = nc.dram_tensor("inb", [2 * 128, B], f32r)
    outb = nc.dram_tensor("outb", [KT * 128, B], f32r, addr_space="Shared")

    rg = [list(range(NCORES))]

    with tile.TileContext(nc) as tc:
        with (
            tc.tile_pool(name="w", bufs=1) as wp,
            tc.tile_pool(name="st", bufs=1) as sp,
            tc.tile_pool(name="ot", bufs=2) as op_,
            tc.tile_pool(name="ps", bufs=6, space="PSUM") as ps,
            tc.tile_pool(name="pso", bufs=2, space="PSUM") as pso,
        ):
            w_eff_sb = wp.tile([128, KT * MT * 128], f32r)
            w_ih_sb = wp.tile([128, MT * 128], f32r)
            w_out_sb = wp.tile([128, KT * D], f32r)
            b0_sb = wp.tile([128, MT], f32)
            beff_sb = wp.tile([128, MT], f32)
            bout_sb = wp.tile([128, 1], f32)
            x0_sb = wp.tile([128, B], f32r)

            hT = sp.tile([128, KT * B], f32r)      # gathered h.T (all cores)
            myh = sp.tile([128, 2 * B], f32r)      # this core's h-slice
            cst = sp.tile([128, 2 * B], f32)       # cell state (2 tiles)
            sig = sp.tile([128, MT * B], f32)      # activated gates
            tnc = sp.tile([128, 2 * B], f32)       # tanh(c)
            tmp = sp.tile([128, 2 * B], f32)

            nc.sync.dma_start(w_eff_sb[:], w_eff[:])
            nc.sync.dma_start(w_ih_sb[:], w_ih[:])
            nc.sync.dma_start(w_out_sb[:], w_out[:])
            nc.sync.dma_start(b0_sb[:], b0[:])
            nc.sync.dma_start(beff_sb[:], beff[:])
            nc.sync.dma_start(bout_sb[:], bout[:])
            nc.sync.dma_start(x0_sb[:], x0t[:])

            def emit_wout(t):
                # out_t = h_t @ W_out.T + b_out, from the gathered hT state.
                po = pso.tile([128, B], f32, tag="po")
                for k in range(KT):
                    nc.tensor.matmul(po[:], w_out_sb[:, k * D:(k + 1) * D],
                                     hT[:, k * B:(k + 1) * B],
                                     start=(k == 0), stop=(k == KT - 1))
                ot = op_.tile([128, B], f32, tag="ot")
                nc.scalar.activation(ot[:], po[:], AF.Identity,
                                     bias=bout_sb[:, 0:1])
                nc.sync.dma_start(out_d[t], ot[:])

            for t in range(t_steps):
                # --- gates for step t (reads hT = gathered h_{t-1}) ---
                for m in range(MT):
                    pt = ps.tile([128, B], f32, tag="g")
                    if t == 0:
                        nc.tensor.matmul(pt[:], w_ih_sb[:, m * 128:(m + 1) * 128],
                                         x0_sb[:], start=True, stop=True)
                    else:
                        for k in range(KT):
                            lhsT = w_eff_sb[:, (k * MT + m) * 128:(k * MT + m + 1) * 128]
                            nc.tensor.matmul(pt[:], lhsT, hT[:, k * B:(k + 1) * B],
                                             start=(k == 0), stop=(k == KT - 1))
                    func = AF.Tanh if m in (4, 5) else AF.Sigmoid
                    bias = (b0_sb if t == 0 else beff_sb)[:, m:m + 1]
                    nc.scalar.activation(sig[:, m * B:(m + 1) * B], pt[:], func,
                                         bias=bias)

                # output projection for the previous step overlaps the gather
                if t > 0:
                    emit_wout(t - 1)

                # --- cell/hidden update ---
                for s in range(2):
                    si = sig[:, (0 + s) * B:(1 + s) * B]
                    sf = sig[:, (2 + s) * B:(3 + s) * B]
                    tg = sig[:, (4 + s) * B:(5 + s) * B]
                    so = sig[:, (6 + s) * B:(7 + s) * B]
                    cs = cst[:, s * B:(s + 1) * B]
                    if t == 0:
                        nc.vector.tensor_mul(cs, si, tg)
                    else:
                        nc.vector.tensor_mul(cs, sf, cs)
                        nc.vector.tensor_mul(tmp[:, s * B:(s + 1) * B], si, tg)
                        nc.vector.tensor_add(cs, cs, tmp[:, s * B:(s + 1) * B])
                    nc.scalar.activation(tnc[:, s * B:(s + 1) * B], cs, AF.Tanh)
                    nc.vector.tensor_mul(myh[:, s * B:(s + 1) * B], so,
                                         tnc[:, s * B:(s + 1) * B])

                # --- AllGather the h-slices into hT ---
                nc.sync.dma_start(
                    inb.ap().rearrange("(s p) n -> p s n", s=2),
                    myh[:].rearrange("p (s n) -> p s n", s=2),
                )
                nc.gpsimd.collective_compute(
                    "AllGather", mybir.AluOpType.bypass, replica_groups=rg,
                    ins=[inb.ap().opt()], outs=[outb.ap().opt()],
                )
                nc.sync.dma_start(
                    hT[:].rearrange("p (k n) -> p k n", k=KT),
                    outb.ap().rearrange("(k p) n -> p k n", k=KT),
                )

            emit_wout(t_steps - 1)

    nc.compile()
    return nc


_BUILDERS = {
    "pipe": _build_pipe,
    "pipe2": _build_pipe2,
    "pipe2_now": lambda t: _build_pipe2(t, delay_w=False),
    "pipe2_noag": lambda t: _build_pipe2(t, no_ag=True),
    "pipe3": _build_pipe3,
    "pipe2s": lambda t: _build_pipe2(t, split_dma=True),
    "pipe4": _build_pipe4,
    "pipe5": _build_pipe5,
    "pipe6": lambda t: _build_pipe5(t, dve_cast=True),
    "flat": _build,
}


def _prep_inputs(tgt, W_ih, W_hh, b_ih, b_hh, W_out, b_out, t_steps,
                 variant=None):
    f32 = np.float32
    if variant is None:
        variant = os.environ.get("LSTM_V", "pipe6")
    tgt = np.asarray(tgt, f32)
    W_ih = np.asarray(W_ih, f32)
    W_hh = np.asarray(W_hh, f32)
    W_out = np.asarray(W_out, f32)
    b = np.asarray(b_ih, f32) + np.asarray(b_hh, f32)
    b_out = np.asarray(b_out, f32)

    W_eff = W_hh + W_ih @ W_out          # [4H, H]
    b_eff = b + W_ih @ b_out             # [4H]

    w_out_arr = np.ascontiguousarray(
        W_out.T.reshape(KT, 128, D).transpose(1, 0, 2).reshape(128, KT * D))
    x0t = np.ascontiguousarray(tgt[:, 0, :].T)          # [128, B]
    bout_arr = np.ascontiguousarray(b_out[:, None])     # [128, 1]

    in_maps = []
    for j in range(NCORES):
        rows = np.concatenate(
            [g * H + j * HS + np.arange(HS) for g in range(4)])
        Wj = W_eff[rows]                                 # [1024, H]
        w_eff_arr = np.ascontiguousarray(
            Wj.T.reshape(KT, 128, MT, 128).transpose(1, 0, 2, 3)
            .reshape(128, KT * MT * 128))
        w_ih_arr = np.ascontiguousarray(W_ih[rows].T)    # [128, 1024]
        b0_arr = np.ascontiguousarray(b[rows].reshape(MT, 128).T)
        beff_arr = np.ascontiguousarray(b_eff[rows].reshape(MT, 128).T)
        in_maps.append({
            "w_eff": w_eff_arr, "w_ih": w_ih_arr, "w_out": w_out_arr,
            "x0t": x0t, "b0": b0_arr, "beff": beff_arr, "bout": bout_arr,
        })
    if variant.startswith("pipe2"):
        import ml_dtypes
        bf16 = ml_dtypes.bfloat16
        for m in in_maps:
            for key in ("w_eff", "w_ih", "w_out", "x0t"):
                m[key] = np.ascontiguousarray(m[key].astype(bf16))
    elif variant == "pipe4":
        import ml_dtypes
        bf16 = ml_dtypes.bfloat16
        for j, m in enumerate(in_maps):
            m["w_own"] = np.ascontiguousarray(
                m["w_out"][:, 2 * j * D:(2 * j + 2) * D].astype(bf16))
            del m["w_out"]
            del m["bout"]
            for key in ("w_eff", "w_ih", "x0t"):
                m[key] = np.ascontiguousarray(m[key].astype(bf16))
    elif variant in ("pipe5", "pipe6"):
        import ml_dtypes
        bf16 = ml_dtypes.bfloat16
        fp8 = ml_dtypes.float8_e4m3
        for j, m in enumerate(in_maps):
            m["w_eff8"] = np.ascontiguousarray(
                (m["w_eff"] * 256.0).astype(fp8))
            m["w_own"] = np.ascontiguousarray(
                m["w_out"][:, 2 * j * D:(2 * j + 2) * D].astype(bf16))
            del m["w_eff"]
            del m["w_out"]
            del m["bout"]
            for key in ("w_ih", "x0t"):
                m[key] = np.ascontiguousarray(m[key].astype(bf16))
    elif variant == "pipe3":
        import ml_dtypes
        bf16 = ml_dtypes.bfloat16
        fp8 = ml_dtypes.float8_e4m3
        WS = 256.0
        for j, m in enumerate(in_maps):
            rows = np.concatenate(
                [g * H + j * HS + np.arange(HS) for g in range(4)])
            WjT = W_eff[rows].T                       # [H, 1024]
            # [k, p, mt, mc] -> [j2, two, p, mt, mc] -> [p, j2, mt, two, mc]
            arr = WjT.reshape(KT // 2, 2, 128, MT, 128)
            arr = np.ascontiguousarray(
                arr.transpose(2, 0, 3, 1, 4).reshape(128, -1))
            m["w_eff8"] = np.ascontiguousarray((arr * WS).astype(fp8))
            del m["w_eff"]
            for key in ("w_ih", "w_out", "x0t"):
                m[key] = np.ascontiguousarray(m[key].astype(bf16))
    return in_maps


def kernel(tgt, W_ih, W_hh, b_ih, b_hh, W_out, b_out):
    from concourse.bass_utils import run_bass_kernel_spmd

    t_steps = int(os.environ.get("LSTM_T", T_FULL))
    variant = os.environ.get("LSTM_V", "pipe6")
    key = (t_steps, variant)
    if key not in _CACHE:
        _CACHE[key] = _BUILDERS[variant](t_steps)
    nc = _CACHE[key]

    in_maps = _prep_inputs(tgt, W_ih, W_hh, b_ih, b_hh, W_out, b_out, t_steps,
                           variant=variant)
    res = run_bass_kernel_spmd(nc, in_maps, core_ids=list(range(NCORES)))
    if variant in ("pipe4", "pipe5", "pipe6"):
        # each core holds the partial projection of its own h-slice
        out = res.results[0]["out"].astype(np.float64)
        for c in range(1, NCORES):
            out += res.results[c]["out"]
        out = (out + np.asarray(b_out, np.float64)[None, :, None]).astype(
            np.float32)                            # [t_steps, D, B]
    else:
        out = res.results[0]["out"]                # [t_steps, D, B]
    full = np.ascontiguousarray(out.transpose(2, 0, 1))  # [B, t_steps, D]
    if t_steps == np.asarray(tgt).shape[1]:
        return full
    # debugging path: pad to full length so callers can slice
    return full

